# revision 24
# baseline (speedup 1.0000x reference)
"""DOA pattern loss kernel for Trainium2 (8 NeuronCores, SPMD).

Computes min_r sum_a (possible_phases[r, a] - phases[a])^2 over a
[1_000_000, 32] codebook, returning the scalar min.

Strategy (memory-bound problem):
  - Shard the codebook rows across 8 cores (125k rows each).
  - Host-side, each core's shard [RC, 32] is split into 4 row-quarters and
    each quarter transposed to [32, QPOS]; quarters stack into a
    [128, QPOS] fp16 layout: partition q = 32*quarter + antenna, free dim
    = row position.  The 32-antenna reduction axis lands on SBUF
    partitions, so no on-device transpose is needed and DMA is fully
    contiguous per partition.  fp16 halves DMA bytes.
  - The free dim streams in tiles with ramp-up/ramp-down widths so compute
    starts as soon as the first small tile lands and the post-DMA tail is
    short.  The whole pipeline is NEGATED (max instead of min) so the
    final cross-partition reduce can use gpsimd partition_all_reduce(max).
  - Squares, split across three engines below the DMA roofline:
      ScalarE route: d2 = Square(x + bias) (bias = -p per partition, fused
        subtract+square, 1 pass); PE sums antenna groups with stationary
        B1 = -group_mask -> psum = -sum d2.
      VectorE + GpSimd routes: one tensor_mul y = x * x; PE: B1^T y
        (= -sum x^2) accumulated with B2^T x (B2 = +2p*mask) -> psum =
        -sum d2 + sum(p^2); the constant is subtracted at the end.
        (GpSimd cannot read PSUM or per-partition scalars, so it squares
        rather than reduces.)
  - Chunks pack into PSUM banks of 4 across tile boundaries (4 matmuls at
    partition offsets 0/32/64/96 -> 8192 distinct row sums per bank).
    VectorE takes a free-dim MAX per retired bank into a staging column
    (issue deferred by one bank so the in-order DVE queue rarely stalls
    on PE).
  - Tail: two free-dim MAX reduces + constant fixup + elementwise max ->
    [128, 1] -> DRAM.  Host takes -max over 8 cores x 128 partitions
    (gpsimd partition_all_reduce was tried and costs ~5us for 128 values).
"""

import numpy as np

P = 128          # SBUF partitions
A = 32           # antennas
NQ = 4           # row-quarters stacked on the partition axis
CHUNK = 512      # matmul rhs free size = one PSUM bank of fp32
NCORES = 8

QPOS = 31250     # row positions per quarter per core
RC = NQ * QPOS   # rows per core = 125000
W = 4096         # cruise DMA tile width (1 MB fp16)

_cache: dict = {}


def _tile_widths(qpos: int, w: int) -> list:
    """Ramp-up, cruise, ramp-down tile widths summing to qpos."""
    small = max(512, w // 2)
    ws = []
    rem = qpos
    for _ in range(2):
        if rem >= small + w:
            ws.append(small)
            rem -= small
    while rem >= 2 * w:
        ws.append(w)
        rem -= w
    cur = w
    while cur >= 1024 and rem > cur:
        if rem >= cur + cur // 2:
            ws.append(cur)
            rem -= cur
        cur //= 2
    ws.append(rem)
    assert sum(ws) == qpos, (ws, qpos)
    return ws


def _split(nch: int) -> tuple:
    """(scalar, dve, gpsimd) chunk counts for one tile."""
    ns = max(0, round(nch * 0.47))
    ng = max(0, round(nch * 0.26))
    nd = nch - ns - ng
    return ns, nd, ng


def build_nc(qpos: int = QPOS, w: int = W, reps: int = 1, debug: bool = False):
    """Build the single-core Bass program (same NEFF runs SPMD on all cores).

    reps > 1 repeats the whole compute loop (timing experiments only).
    """
    from contextlib import ExitStack

    import concourse.bacc as bacc
    import concourse.tile as tile
    from concourse import bass_isa, mybir

    dt = mybir.dt.float16
    f32 = mybir.dt.float32
    nc = bacc.Bacc("TRN2", target_bir_lowering=False)

    cb = nc.dram_tensor("cb", [P, qpos], dt, kind="ExternalInput")
    # consts: col0 = -p (ACT bias), col1 = -sum(p^2)
    consts = nc.dram_tensor("consts", [P, 2], f32, kind="ExternalInput")
    b1 = nc.dram_tensor("b1", [P, A], dt, kind="ExternalInput")   # -group mask
    b2 = nc.dram_tensor("b2", [P, A], dt, kind="ExternalInput")   # +2p * mask
    out = nc.dram_tensor("out", [P, 1], f32, kind="ExternalOutput")
    if debug:
        dbg_fs = nc.dram_tensor("dbg_fs", [P, 2], f32, kind="ExternalOutput")

    widths = _tile_widths(qpos, w)
    offs = []
    o = 0
    for wt in widths:
        offs.append((o, wt))
        o += wt

    # stage columns: one per PSUM bank, generous upper bound
    total_chunks = sum(wt // CHUNK for _, wt in offs) + sum(
        1 for _, wt in offs if wt % CHUNK
    )
    n_banks = (total_chunks // 4 + 2 * len(offs) + 4) * reps

    NEGBIG = -3.0e38  # -inf stand-in for the max staging columns

    with tile.TileContext(nc) as tc:
        with ExitStack() as ctx:
            singles = ctx.enter_context(tc.tile_pool(name="singles", bufs=1))
            xpool = ctx.enter_context(tc.tile_pool(name="xin", bufs=8))
            ypool = ctx.enter_context(tc.tile_pool(name="yv", bufs=6))
            ppool = ctx.enter_context(tc.tile_pool(name="ps", bufs=8, space="PSUM"))

            consts_s = singles.tile([P, 2], f32)
            nc.scalar.dma_start(out=consts_s[:, :], in_=consts[:, :])
            b1_s = singles.tile([P, A], dt)
            nc.scalar.dma_start(out=b1_s[:, :], in_=b1[:, :])
            b2_s = singles.tile([P, A], dt)
            nc.scalar.dma_start(out=b2_s[:, :], in_=b2[:, :])

            stage_s = singles.tile([P, n_banks], f32)
            stage_d = singles.tile([P, n_banks], f32)
            nc.gpsimd.memset(stage_s[:, :], NEGBIG)
            nc.gpsimd.memset(stage_d[:, :], NEGBIG)
            fs = singles.tile([P, 1], f32)
            fd = singles.tile([P, 1], f32)
            warm = singles.tile([P, 1], f32)
            # dummy Square: pulls ACT_TABLE_LOAD into the DMA ramp
            nc.scalar.activation(
                warm[:, :],
                consts_s[:, :1],
                mybir.ActivationFunctionType.Square,
                scale=0.0,
            )

            state = {
                "open": {"s": None, "d": None},  # route -> (ps, [chunk...])
                "col": {"s": 0, "d": 0},
                "ready": [],  # retired banks whose DVE max is not yet issued
            }

            def flush_ready(keep: int = 1):
                while len(state["ready"]) > keep:
                    route, ps, npart, cw = state["ready"].pop(0)
                    stg = stage_s if route == "s" else stage_d
                    col = state["col"][route]
                    state["col"][route] += 1
                    nc.vector.tensor_reduce(
                        out=stg[:npart, col : col + 1],
                        in_=ps[:npart, :cw],
                        axis=mybir.AxisListType.X,
                        op=mybir.AluOpType.max,
                    )

            def retire(route):
                bank = state["open"][route]
                state["open"][route] = None
                if bank is None:
                    return
                ps, chunks = bank
                if route == "s":
                    # chunks hold d2 >= 0; psum = B1^T d2 = -sum d2
                    for jj, (reg, cw) in enumerate(chunks):
                        nc.tensor.matmul(
                            ps[32 * jj : 32 * (jj + 1), :cw],
                            b1_s[:, :],
                            reg,
                            start=True,
                            stop=True,
                            tile_position=(0, 32 * jj),
                        )
                else:
                    # y chunks: psum = B1^T y + B2^T x = -sum d2 + sum p^2
                    for jj, (reg, xreg, cw) in enumerate(chunks):
                        nc.tensor.matmul(
                            ps[32 * jj : 32 * (jj + 1), :cw],
                            b1_s[:, :],
                            reg,
                            start=True,
                            stop=False,
                            tile_position=(0, 32 * jj),
                        )
                    for jj, (reg, xreg, cw) in enumerate(chunks):
                        nc.tensor.matmul(
                            ps[32 * jj : 32 * (jj + 1), :cw],
                            b2_s[:, :],
                            xreg,
                            start=False,
                            stop=True,
                            tile_position=(0, 32 * jj),
                        )
                cw = max(c[-1] for c in chunks)
                state["ready"].append((route, ps, 32 * len(chunks), cw))
                flush_ready()

            def push(route, *args):
                if state["open"][route] is None:
                    ps = ppool.tile([P, CHUNK], f32, tag="ps")
                    state["open"][route] = (ps, [])
                state["open"][route][1].append(args)
                if len(state["open"][route][1]) == 4:
                    retire(route)

            all_offs = offs * reps
            for ti, (o, wt) in enumerate(all_offs):
                x = xpool.tile([P, w], dt, tag="x")
                nc.sync.dma_start(out=x[:, :wt], in_=cb[:, o : o + wt])

                nch = wt // CHUNK          # full 512-wide chunks
                tail = wt - nch * CHUNK    # ragged tail chunk
                ns, nd, ng = _split(nch)
                if ti >= len(all_offs) - 3:
                    # keep slow GpSimd chunks off the post-DMA critical tail;
                    # DVE muls are the cheapest per chunk
                    nd += ng
                    ng = 0
                aw = ns * CHUNK
                if aw:
                    # in-place d2 over x (elementwise)
                    nc.scalar.activation(
                        x[:, :aw],
                        x[:, :aw],
                        mybir.ActivationFunctionType.Square,
                        bias=consts_s[:, :1],
                        scale=1.0,
                    )
                if tail:
                    nc.scalar.activation(
                        x[:, nch * CHUNK : wt],
                        x[:, nch * CHUNK : wt],
                        mybir.ActivationFunctionType.Square,
                        bias=consts_s[:, :1],
                        scale=1.0,
                    )
                y = None
                if nd + ng:
                    y = ypool.tile([P, w], dt, tag="y")
                    # single pass y = x * x (Pool has no per-partition-scalar
                    # op, so both engines square and PE applies the -2p term)
                    if nd:
                        dw = nd * CHUNK
                        nc.vector.tensor_mul(
                            y[:, aw : aw + dw],
                            x[:, aw : aw + dw],
                            x[:, aw : aw + dw],
                        )
                    if ng:
                        g0 = aw + nd * CHUNK
                        gw = ng * CHUNK
                        nc.gpsimd.tensor_mul(
                            y[:, g0 : g0 + gw],
                            x[:, g0 : g0 + gw],
                            x[:, g0 : g0 + gw],
                        )

                for c in range(ns):
                    push("s", x[:, c * CHUNK : (c + 1) * CHUNK], CHUNK)
                if tail:
                    # ragged chunk gets a private bank: a shared bank would
                    # feed garbage PSUM columns into the staged max
                    retire("s")
                    push("s", x[:, nch * CHUNK : wt], tail)
                    retire("s")
                for c in range(ns, nch):
                    push(
                        "d",
                        y[:, c * CHUNK : (c + 1) * CHUNK],
                        x[:, c * CHUNK : (c + 1) * CHUNK],
                        CHUNK,
                    )

            retire("s")
            retire("d")
            flush_ready(keep=0)
            assert max(state["col"].values()) <= n_banks, (state["col"], n_banks)

            # fs = max over free of stage_s = -min d2 (scalar chunks)
            nc.vector.tensor_reduce(
                out=fs[:, :], in_=stage_s[:, :],
                axis=mybir.AxisListType.X, op=mybir.AluOpType.max,
            )
            # fd = max over free of stage_d, then subtract sum(p^2)
            nc.vector.tensor_reduce(
                out=fd[:, :], in_=stage_d[:, :],
                axis=mybir.AxisListType.X, op=mybir.AluOpType.max,
            )
            nc.vector.tensor_scalar_add(fd[:, :], fd[:, :], consts_s[:, 1:2])
            if debug:
                dbg = singles.tile([P, 2], f32)
                nc.vector.tensor_copy(dbg[:, 0:1], fs[:, :])
                nc.vector.tensor_copy(dbg[:, 1:2], fd[:, :])
                nc.sync.dma_start(out=dbg_fs[:, :], in_=dbg[:, :])
            nc.vector.tensor_max(fs[:, :], fs[:, :], fd[:, :])
            nc.sync.dma_start(out=out[:, :], in_=fs[:, :])

    nc.compile()
    return nc


def make_in_maps(possible_phases: np.ndarray, phases: np.ndarray, qpos: int = QPOS):
    """Shard + quarter-transpose the codebook; build per-core input maps."""
    rc = NQ * qpos
    rpad = NCORES * rc
    pp = np.asarray(possible_phases, dtype=np.float32).astype(np.float16)
    r = pp.shape[0]
    assert rpad >= r and rpad - r <= r, (rpad, r)
    if rpad > r:
        # pad with duplicate rows: the min is unchanged
        pp = np.concatenate([pp, pp[: rpad - r]], axis=0)

    ph = np.asarray(phases, dtype=np.float32).reshape(A)
    consts = np.stack(
        [
            np.tile(-ph, NQ),
            np.full(P, -float((ph.astype(np.float64) ** 2).sum()), np.float32),
        ],
        axis=1,
    ).astype(np.float32)  # [128, 2]
    mask = np.kron(
        np.eye(NQ, dtype=np.float32), np.ones((A, A // NQ), np.float32)
    )  # [128, 32], 1 iff q//32 == m//8
    b1 = (-mask).astype(np.float16)
    b2 = (mask * (2.0 * np.tile(ph, NQ))[:, None]).astype(np.float16)

    in_maps = []
    for c in range(NCORES):
        shard = pp[c * rc : (c + 1) * rc]  # [rc, 32]
        cbq = np.ascontiguousarray(
            shard.reshape(NQ, qpos, A).transpose(0, 2, 1).reshape(P, qpos)
        )
        in_maps.append({"cb": cbq, "consts": consts, "b1": b1, "b2": b2})
    return in_maps


def kernel(possible_phases: np.ndarray, phases: np.ndarray) -> np.ndarray:
    from concourse.bass_utils import run_bass_kernel_spmd

    if "nc" not in _cache:
        _cache["nc"] = build_nc()
    in_maps = make_in_maps(possible_phases, phases)
    res = run_bass_kernel_spmd(_cache["nc"], in_maps, core_ids=list(range(NCORES)))
    negmins = np.stack([res.results[c]["out"] for c in range(NCORES)])
    return np.float32(-negmins.max())  # out[p] = -min over that partition's banks


# revision 32
# speedup vs baseline: 1.1433x; 1.1433x over previous
"""DOA pattern loss kernel for Trainium2 (8 NeuronCores, SPMD).

Computes min_r sum_a (possible_phases[r, a] - phases[a])^2 over a
[1_000_000, 32] codebook, returning the scalar min.

Strategy (memory-bound problem):
  - Shard the codebook rows across 8 cores (125k rows each).
  - Host-side, each core's shard [RC, 32] is split into 4 row-quarters and
    each quarter transposed to [32, QPOS]; quarters stack into a
    [128, QPOS] fp16 layout: partition q = 32*quarter + antenna, free dim
    = row position.  The 32-antenna reduction axis lands on SBUF
    partitions, so no on-device transpose is needed and DMA is fully
    contiguous per partition.  fp16 halves DMA bytes.
  - The free dim streams in tiles with ramp-up/ramp-down widths so compute
    starts as soon as the first small tile lands and the post-DMA tail is
    short.  The whole pipeline is NEGATED (max instead of min) so the
    final cross-partition reduce can use gpsimd partition_all_reduce(max).
  - Squares, split across three engines below the DMA roofline:
      ScalarE route: d2 = Square(x + bias) (bias = -p per partition, fused
        subtract+square, 1 pass); PE sums antenna groups with stationary
        B1 = -group_mask -> psum = -sum d2.
      VectorE + GpSimd routes: one tensor_mul y = x * x; PE: B1^T y
        (= -sum x^2) accumulated with B2^T x (B2 = +2p*mask) -> psum =
        -sum d2 + sum(p^2); the constant is subtracted at the end.
        (GpSimd cannot read PSUM or per-partition scalars, so it squares
        rather than reduces.)
  - Chunks pack into PSUM banks of 4 across tile boundaries (4 matmuls at
    partition offsets 0/32/64/96 -> 8192 distinct row sums per bank).
    VectorE takes a free-dim MAX per retired bank into a staging column
    (issue deferred by one bank so the in-order DVE queue rarely stalls
    on PE).
  - Tail: two free-dim MAX reduces + constant fixup + elementwise max ->
    [128, 1], gpsimd partition_all_reduce(max) -> [1, 1] -> single
    4-byte-descriptor DMA out.  Host negates and mins over 8 cores.
"""

import numpy as np

P = 128          # SBUF partitions
A = 32           # antennas
NQ = 4           # row-quarters stacked on the partition axis
CHUNK = 512      # matmul rhs free size = one PSUM bank of fp32
NCORES = 8

QPOS = 31250     # row positions per quarter per core
RC = NQ * QPOS   # rows per core = 125000
W = 4096         # cruise DMA tile width (1 MB fp16)

_cache: dict = {}


def _tile_widths(qpos: int, w: int) -> list:
    """Ramp-up, cruise, ramp-down tile widths summing to qpos."""
    small = max(512, w // 2)
    ws = []
    rem = qpos
    for _ in range(2):
        if rem >= small + w:
            ws.append(small)
            rem -= small
    while rem >= 2 * w:
        ws.append(w)
        rem -= w
    cur = w
    while cur >= 1024 and rem > cur:
        if rem >= cur + cur // 2:
            ws.append(cur)
            rem -= cur
        cur //= 2
    ws.append(rem)
    assert sum(ws) == qpos, (ws, qpos)
    return ws


def _split(nch: int, ti: int = 0) -> tuple:
    """(scalar, dve, gpsimd) chunk counts for one tile."""
    ns = max(0, round(nch * 0.47))
    ng = max(0, round(nch * 0.26))
    nd = nch - ns - ng
    return ns, nd, ng


def build_nc(qpos: int = QPOS, w: int = W, reps: int = 1, debug: bool = False):
    """Build the single-core Bass program (same NEFF runs SPMD on all cores).

    reps > 1 repeats the whole compute loop (timing experiments only).
    """
    from contextlib import ExitStack

    import concourse.bacc as bacc
    import concourse.tile as tile
    from concourse import bass_isa, mybir

    dt = mybir.dt.float16
    f32 = mybir.dt.float32
    nc = bacc.Bacc("TRN2", target_bir_lowering=False)

    cb = nc.dram_tensor("cb", [P, qpos], dt, kind="ExternalInput")
    # consts: col0 = -p (ACT bias), col1 = -sum(p^2)
    consts = nc.dram_tensor("consts", [P, 2], f32, kind="ExternalInput")
    b1 = nc.dram_tensor("b1", [P, A], dt, kind="ExternalInput")   # -group mask
    b2 = nc.dram_tensor("b2", [P, A], dt, kind="ExternalInput")   # +2p * mask
    out = nc.dram_tensor("out", [1, 1], f32, kind="ExternalOutput")
    if debug:
        dbg_fs = nc.dram_tensor("dbg_fs", [P, 2], f32, kind="ExternalOutput")

    widths = _tile_widths(qpos, w)
    offs = []
    o = 0
    for wt in widths:
        offs.append((o, wt))
        o += wt

    # stage columns: one per PSUM bank, generous upper bound
    total_chunks = sum(wt // CHUNK for _, wt in offs) + sum(
        1 for _, wt in offs if wt % CHUNK
    )
    n_banks = (total_chunks // 4 + 2 * len(offs) + 4) * reps

    NEGBIG = -3.0e38  # -inf stand-in for the max staging columns

    with tile.TileContext(nc) as tc:
        with ExitStack() as ctx:
            singles = ctx.enter_context(tc.tile_pool(name="singles", bufs=1))
            xpool = ctx.enter_context(tc.tile_pool(name="xin", bufs=8))
            ypool = ctx.enter_context(tc.tile_pool(name="yv", bufs=6))
            ppool = ctx.enter_context(tc.tile_pool(name="ps", bufs=8, space="PSUM"))

            consts_s = singles.tile([P, 2], f32)
            nc.scalar.dma_start(out=consts_s[:, :], in_=consts[:, :])
            b1_s = singles.tile([P, A], dt)
            nc.scalar.dma_start(out=b1_s[:, :], in_=b1[:, :])
            b2_s = singles.tile([P, A], dt)
            nc.scalar.dma_start(out=b2_s[:, :], in_=b2[:, :])

            stage_s = singles.tile([P, n_banks], f32)
            stage_d = singles.tile([P, n_banks], f32)
            nc.gpsimd.memset(stage_s[:, :], NEGBIG)
            nc.gpsimd.memset(stage_d[:, :], NEGBIG)
            fs = singles.tile([P, 1], f32)
            fd = singles.tile([P, 1], f32)
            ored = singles.tile([P, 1], f32)
            warm = singles.tile([P, 1], f32)
            # dummy Square: pulls ACT_TABLE_LOAD into the DMA ramp
            nc.scalar.activation(
                warm[:, :],
                consts_s[:, :1],
                mybir.ActivationFunctionType.Square,
                scale=0.0,
            )

            state = {
                "open": {"s": None, "d": None},  # route -> (ps, [chunk...])
                "col": {"s": 0, "d": 0},
                "ready": [],  # retired banks whose DVE max is not yet issued
            }

            def flush_ready(keep: int = 1):
                while len(state["ready"]) > keep:
                    route, ps, npart, cw = state["ready"].pop(0)
                    stg = stage_s if route == "s" else stage_d
                    col = state["col"][route]
                    state["col"][route] += 1
                    nc.vector.tensor_reduce(
                        out=stg[:npart, col : col + 1],
                        in_=ps[:npart, :cw],
                        axis=mybir.AxisListType.X,
                        op=mybir.AluOpType.max,
                    )

            def retire(route):
                bank = state["open"][route]
                state["open"][route] = None
                if bank is None:
                    return
                ps, chunks = bank
                if route == "s":
                    # chunks hold d2 >= 0; psum = B1^T d2 = -sum d2
                    for jj, (reg, cw) in enumerate(chunks):
                        nc.tensor.matmul(
                            ps[32 * jj : 32 * (jj + 1), :cw],
                            b1_s[:, :],
                            reg,
                            start=True,
                            stop=True,
                            tile_position=(0, 32 * jj),
                        )
                else:
                    # y chunks: psum = B1^T y + B2^T x = -sum d2 + sum p^2
                    for jj, (reg, xreg, cw) in enumerate(chunks):
                        nc.tensor.matmul(
                            ps[32 * jj : 32 * (jj + 1), :cw],
                            b1_s[:, :],
                            reg,
                            start=True,
                            stop=False,
                            tile_position=(0, 32 * jj),
                        )
                    for jj, (reg, xreg, cw) in enumerate(chunks):
                        nc.tensor.matmul(
                            ps[32 * jj : 32 * (jj + 1), :cw],
                            b2_s[:, :],
                            xreg,
                            start=False,
                            stop=True,
                            tile_position=(0, 32 * jj),
                        )
                cw = max(c[-1] for c in chunks)
                state["ready"].append((route, ps, 32 * len(chunks), cw))
                flush_ready()

            def push(route, *args):
                if state["open"][route] is None:
                    ps = ppool.tile([P, CHUNK], f32, tag="ps")
                    state["open"][route] = (ps, [])
                state["open"][route][1].append(args)
                if len(state["open"][route][1]) == 4:
                    retire(route)

            all_offs = offs * reps
            for ti, (o, wt) in enumerate(all_offs):
                x = xpool.tile([P, w], dt, tag="x")
                nc.sync.dma_start(out=x[:, :wt], in_=cb[:, o : o + wt])

                nch = wt // CHUNK          # full 512-wide chunks
                tail = wt - nch * CHUNK    # ragged tail chunk
                ns, nd, ng = _split(nch, ti)
                if ti >= len(all_offs) - 3:
                    # keep slow GpSimd chunks off the post-DMA critical tail;
                    # DVE muls are the cheapest per chunk
                    nd += ng
                    ng = 0
                aw = ns * CHUNK
                if aw:
                    # in-place d2 over x (elementwise)
                    nc.scalar.activation(
                        x[:, :aw],
                        x[:, :aw],
                        mybir.ActivationFunctionType.Square,
                        bias=consts_s[:, :1],
                        scale=1.0,
                    )
                if tail:
                    nc.scalar.activation(
                        x[:, nch * CHUNK : wt],
                        x[:, nch * CHUNK : wt],
                        mybir.ActivationFunctionType.Square,
                        bias=consts_s[:, :1],
                        scale=1.0,
                    )
                y = None
                if nd + ng:
                    y = ypool.tile([P, w], dt, tag="y")
                    # single pass y = x * x (Pool has no per-partition-scalar
                    # op, so both engines square and PE applies the -2p term)
                    if nd:
                        dw = nd * CHUNK
                        nc.vector.tensor_mul(
                            y[:, aw : aw + dw],
                            x[:, aw : aw + dw],
                            x[:, aw : aw + dw],
                        )
                    if ng:
                        g0 = aw + nd * CHUNK
                        gw = ng * CHUNK
                        nc.gpsimd.tensor_mul(
                            y[:, g0 : g0 + gw],
                            x[:, g0 : g0 + gw],
                            x[:, g0 : g0 + gw],
                        )

                for c in range(ns):
                    push("s", x[:, c * CHUNK : (c + 1) * CHUNK], CHUNK)
                if tail:
                    # ragged chunk gets a private bank: a shared bank would
                    # feed garbage PSUM columns into the staged max
                    retire("s")
                    push("s", x[:, nch * CHUNK : wt], tail)
                    retire("s")
                for c in range(ns, nch):
                    push(
                        "d",
                        y[:, c * CHUNK : (c + 1) * CHUNK],
                        x[:, c * CHUNK : (c + 1) * CHUNK],
                        CHUNK,
                    )

            retire("s")
            retire("d")
            flush_ready(keep=0)
            assert max(state["col"].values()) <= n_banks, (state["col"], n_banks)

            # fs = max over free of stage_s = -min d2 (scalar chunks)
            nc.vector.tensor_reduce(
                out=fs[:, :], in_=stage_s[:, :],
                axis=mybir.AxisListType.X, op=mybir.AluOpType.max,
            )
            # fd = max over free of stage_d, then subtract sum(p^2)
            nc.vector.tensor_reduce(
                out=fd[:, :], in_=stage_d[:, :],
                axis=mybir.AxisListType.X, op=mybir.AluOpType.max,
            )
            nc.vector.tensor_scalar_add(fd[:, :], fd[:, :], consts_s[:, 1:2])
            if debug:
                dbg = singles.tile([P, 2], f32)
                nc.vector.tensor_copy(dbg[:, 0:1], fs[:, :])
                nc.vector.tensor_copy(dbg[:, 1:2], fd[:, :])
                nc.sync.dma_start(out=dbg_fs[:, :], in_=dbg[:, :])
            nc.vector.tensor_max(fs[:, :], fs[:, :], fd[:, :])
            # cross-partition max on gpsimd (~1.6us) + a single-descriptor
            # 4-byte DMA out: ends the NEFF's useful window promptly
            nc.gpsimd.partition_all_reduce(
                ored[:, :], fs[:, :], 128, bass_isa.ReduceOp.max
            )
            nc.sync.dma_start(out=out[:, :], in_=ored[:1, :])

    nc.compile()
    return nc


def make_in_maps(possible_phases: np.ndarray, phases: np.ndarray, qpos: int = QPOS):
    """Shard + quarter-transpose the codebook; build per-core input maps."""
    rc = NQ * qpos
    rpad = NCORES * rc
    pp = np.asarray(possible_phases, dtype=np.float32).astype(np.float16)
    r = pp.shape[0]
    assert rpad >= r and rpad - r <= r, (rpad, r)
    if rpad > r:
        # pad with duplicate rows: the min is unchanged
        pp = np.concatenate([pp, pp[: rpad - r]], axis=0)

    ph = np.asarray(phases, dtype=np.float32).reshape(A)
    consts = np.stack(
        [
            np.tile(-ph, NQ),
            np.full(P, -float((ph.astype(np.float64) ** 2).sum()), np.float32),
        ],
        axis=1,
    ).astype(np.float32)  # [128, 2]
    mask = np.kron(
        np.eye(NQ, dtype=np.float32), np.ones((A, A // NQ), np.float32)
    )  # [128, 32], 1 iff q//32 == m//8
    b1 = (-mask).astype(np.float16)
    b2 = (mask * (2.0 * np.tile(ph, NQ))[:, None]).astype(np.float16)

    in_maps = []
    for c in range(NCORES):
        shard = pp[c * rc : (c + 1) * rc]  # [rc, 32]
        cbq = np.ascontiguousarray(
            shard.reshape(NQ, qpos, A).transpose(0, 2, 1).reshape(P, qpos)
        )
        in_maps.append({"cb": cbq, "consts": consts, "b1": b1, "b2": b2})
    return in_maps


def kernel(possible_phases: np.ndarray, phases: np.ndarray) -> np.ndarray:
    from concourse.bass_utils import run_bass_kernel_spmd

    if "nc" not in _cache:
        _cache["nc"] = build_nc()
    in_maps = make_in_maps(possible_phases, phases)
    res = run_bass_kernel_spmd(_cache["nc"], in_maps, core_ids=list(range(NCORES)))
    negmins = np.stack([res.results[c]["out"] for c in range(NCORES)])
    return np.float32(-negmins.max())  # out = -min of that core's shard
